# revision 18
# baseline (speedup 1.0000x reference)
"""DilatedCNN forward on 8 TRN2 NeuronCores.

Strategy: data-parallel over the sequence dim N with halo. Each core owns
M=1024 rows plus an 8-row halo on each side (8 = sum of dilations
[1,2,4,1]); with the halo, all four layers are computed fully locally —
no collectives. The activation state lives in SBUF *transposed*
(feature-major: [128 partitions = feature chunk, rows in the free dim]) so
that
  * the concat [X, X_left, X_right] is just three column-shifted views of
    the same buffer (shifts along the free dim are free),
  * the 3072-feature contraction has features on partitions as the
    TensorEngine requires for both operands,
  * each layer's output is again feature-major — ready to be the next
    layer's input with no data movement,
  * the per-feature bias is a per-partition scalar for the activation op.

All matmul operands are bf16 (HW-measured: 147ns/instr at nb=346 vs
159ns for fp32r; LDWEIGHTS halves to 97ns so it stays hidden). The
state is *stored* bf16 and ping-pongs between two buffers per layer
(S0 -> S1 -> S0 -> S1), so no per-layer rounding copy is needed and
epilogue writes never hazard against the same layer's matmul reads.
End-to-end rel err vs the fp32 reference: ~3e-3 (gate 2e-2).

DMA is packet-rate-bound (~600ns minimum for any 128-row transfer), so
weights are laid out partition-major on the host ([l, h, 128, KT, 512])
and each half-layer loads in 2-4 large-row chunks instead of 24
per-tile transfers. Weight SBUF double-buffers by layer parity; each
layer's tiles stream in a whole layer ahead of use.

Accumulation groups are issued as a rolling 2-deep software pipeline
(group i's k12..k23 interleave with group i+1's k0..k11), hiding the
one-slot PE drain at every accumulation-group boundary. The final layer
runs feature-tile-major and writes fp32 results to a staging buffer
whose 512-row chunks DMA out as each epilogue lands.

Out-of-range rows are refreshed with the `oob` vector between layers via
copy_predicated driven by per-core mask/fill inputs, so all 8 cores run
one identical program.
"""

import numpy as np
import ml_dtypes

import concourse.bacc as bacc
import concourse.mybir as mybir
import concourse.tile as tile
from concourse.bass_utils import run_bass_kernel_spmd

N, DIM, NL = 8192, 1024, 4
NCORES = 8
M = N // NCORES           # rows per core
H = 8                     # halo rows each side (sum of dilations)
PAD = 4                   # zero cols so shifted reads stay in-bounds
B = M + 2 * H             # 1040 buffer rows
FB = PAD + B + PAD        # 1048 free-dim cols of the state buffer
DIL = [1, 2, 4, 1]
KT = 3 * DIM // 128       # 24 contraction tiles
DT = DIM // 128           # 8 feature tiles
# Per-layer compute windows (rows [start, start+size) of the B-row buffer),
# shrinking by the dilation each layer.
ROW_BLOCKS_L = [
    [(1, 346), (347, 346), (693, 346)],   # layer 1: rows [1, 1039)
    [(3, 346), (349, 344), (693, 344)],   # layer 2: rows [3, 1037)
    [(7, 342), (349, 342), (691, 342)],   # layer 3: rows [7, 1033)
    [(8, 512), (520, 512)],               # layer 4: rows [8, 1032)
]
F32 = mybir.dt.float32
BF16 = mybir.dt.bfloat16

# Contraction order, dt-grouped: slot j contracts k-tile PERM[j], so the
# k-loop touches input slab dt only every third slot — X slab dt_i is
# first needed at slot 3i, tripling the head's supply slack. PSUM
# accumulation is order-agnostic. The host lays the weight k-axis out in
# this order, so slot j reads weight row j and chunked weight DMAs
# arrive exactly in consumption order.
PERM = [(j % 3) * DT + j // 3 for j in range(KT)]

_CACHE = {}
LAST_RESULTS = None  # test harness reads exec_time_ns from here


def _build():
    nc = bacc.Bacc("TRN2", target_bir_lowering=False, debug=False)

    xs_d = nc.dram_tensor("XST", [128, DT, B], BF16, kind="ExternalInput")
    w_d = nc.dram_tensor("WT", [NL, 2, 4, 128, KT, 128], BF16,
                         kind="ExternalInput")
    b_d = nc.dram_tensor("BS", [128, NL * DT], F32, kind="ExternalInput")
    ml_d = nc.dram_tensor("ML", [128, DT, H], mybir.dt.uint8, kind="ExternalInput")
    fl_d = nc.dram_tensor("FL", [128, DT, H], BF16, kind="ExternalInput")
    mr_d = nc.dram_tensor("MR", [128, DT, H], mybir.dt.uint8, kind="ExternalInput")
    fr_d = nc.dram_tensor("FR", [128, DT, H], BF16, kind="ExternalInput")
    y_d = nc.dram_tensor("YT", [128, DT, M], F32, kind="ExternalOutput")

    with tile.TileContext(nc) as tc:
        with (
            tc.tile_pool(name="state", bufs=1) as state_pool,
            tc.tile_pool(name="wpool", bufs=1) as w_pool,
            tc.tile_pool(name="const", bufs=1) as const_pool,
            tc.tile_pool(name="tmp", bufs=4) as tmp_pool,
            tc.tile_pool(name="gps", bufs=6, space="PSUM") as gps_pool,
        ):
            S0 = state_pool.tile([128, DT, FB], BF16)  # even layers' input
            S1 = state_pool.tile([128, DT, FB], BF16)  # odd layers' input
            YS = state_pool.tile([128, DT, M], F32)    # final fp32 output

            # zero the PAD columns once; epilogues never touch them
            for Sb in (S0, S1):
                nc.gpsimd.memset(Sb[:, :, 0:PAD], 0.0)
                nc.gpsimd.memset(Sb[:, :, PAD + B:FB], 0.0)

            # ---- input prologue. All DMA queues share one AXI port
            # (~310 GB/s aggregate), and the sync/scalar queues are
            # hardware-DGE with ~one transfer in flight (each DMA trickles
            # out 2-4us apart), while gpsimd's software-DGE queue pipelines
            # at full port speed. So: everything head-critical rides gpsimd
            # in exact first-need order (first weight block's chunks
            # interleaved with the X slabs); sync/scalar only carry one
            # early X slab each plus the tiny consts.
            w_tiles = {}
            for l in range(NL):
                for h in range(2):
                    for mtl in range(4):
                        w_tiles[(l, h, mtl)] = w_pool.tile(
                            [128, KT, 128], BF16,
                            tag=f"w{l % 2}_{h}_{mtl}",
                            name=f"w{l}_{h}_{mtl}")

            nc.sync.dma_start(S0[:, 0, PAD:PAD + B], xs_d[:, 0, :])
            nc.scalar.dma_start(S0[:, 1, PAD:PAD + B], xs_d[:, 1, :])

            w000 = w_tiles[(0, 0, 0)]
            gp_seq = [
                lambda: nc.gpsimd.dma_start(w000[:, 0:8, :],
                                            w_d[0, 0, 0, :, 0:8, :]),
                lambda: nc.gpsimd.dma_start(S0[:, 2, PAD:PAD + B],
                                            xs_d[:, 2, :]),
                lambda: nc.gpsimd.dma_start(S0[:, 3, PAD:PAD + B],
                                            xs_d[:, 3, :]),
                lambda: nc.gpsimd.dma_start(w000[:, 8:16, :],
                                            w_d[0, 0, 0, :, 8:16, :]),
                lambda: nc.gpsimd.dma_start(S0[:, 4, PAD:PAD + B],
                                            xs_d[:, 4, :]),
                lambda: nc.gpsimd.dma_start(S0[:, 5, PAD:PAD + B],
                                            xs_d[:, 5, :]),
                lambda: nc.gpsimd.dma_start(w000[:, 16:KT, :],
                                            w_d[0, 0, 0, :, 16:KT, :]),
                lambda: nc.gpsimd.dma_start(S0[:, 6, PAD:PAD + B],
                                            xs_d[:, 6, :]),
                lambda: nc.gpsimd.dma_start(S0[:, 7, PAD:PAD + B],
                                            xs_d[:, 7, :]),
            ]
            for f in gp_seq:
                f()

            bs_t = const_pool.tile([128, NL * DT], F32)
            mask_l = const_pool.tile([128, DT, H], mybir.dt.uint8)
            fill_l = const_pool.tile([128, DT, H], BF16)
            mask_r = const_pool.tile([128, DT, H], mybir.dt.uint8)
            fill_r = const_pool.tile([128, DT, H], BF16)
            nc.scalar.dma_start(bs_t[:], b_d[:])
            nc.scalar.dma_start(mask_l[:], ml_d[:])
            nc.scalar.dma_start(fill_l[:], fl_d[:])
            nc.scalar.dma_start(mask_r[:], mr_d[:])
            nc.scalar.dma_start(fill_r[:], fr_d[:])

            # Remaining weight blocks stream on gpsimd in need order.
            # Weights double-buffer by layer parity ([128, KT, 128] per
            # (l, h, mtl), 6KB DRAM rows): layer l+1's tiles reuse layer
            # l-1's tags, so each load only waits on matmuls a whole layer
            # back and streams in during layer l.
            for l in range(NL):
                for h in range(2):
                    for mtl in range(4):
                        if (l, h, mtl) == (0, 0, 0):
                            continue
                        nc.gpsimd.dma_start(w_tiles[(l, h, mtl)][:],
                                            w_d[l, h, mtl])

            # ---- layers ----
            y_eng = [nc.sync, nc.scalar, nc.gpsimd]
            y_cnt = [0]
            for l, d in enumerate(DIL):
                last = l == NL - 1
                S_in = S0 if l % 2 == 0 else S1
                S_out = S1 if l % 2 == 0 else S0

                def mm_g(ps, c0, nb, mtl, h, j):
                    kt = PERM[j]
                    dt = kt % DT
                    grp = kt // DT
                    sh = 0 if grp == 0 else (-d if grp == 1 else d)
                    nc.tensor.matmul(
                        ps[:, 0:nb],
                        w_tiles[(l, h, mtl)][:, j, :],
                        S_in[:, dt, PAD + c0 + sh:PAD + c0 + sh + nb],
                        start=(j == 0),
                        stop=(j == KT - 1),
                    )

                def epilogue_g(ps, c0, nb, mtl, h):
                    mt = h * 4 + mtl
                    tmp = tmp_pool.tile([128, 512], F32, tag="tmp",
                                        name=f"tmp{l}_{h}_{c0}_{mtl}")
                    nc.scalar.activation(
                        tmp[:, 0:nb],
                        ps[:, 0:nb],
                        mybir.ActivationFunctionType.Relu,
                        bias=bs_t[:, l * DT + mt:l * DT + mt + 1],
                        scale=0.5,
                    )
                    # out = 0.5*S_in + relu(0.5*cat@W + 0.5*b)
                    if last:
                        out_ap = YS[:, mt, c0 - H:c0 - H + nb]
                    else:
                        out_ap = S_out[:, mt, PAD + c0:PAD + c0 + nb]
                    nc.vector.scalar_tensor_tensor(
                        out_ap,
                        S_in[:, mt, PAD + c0:PAD + c0 + nb],
                        0.5,
                        tmp[:, 0:nb],
                        mybir.AluOpType.mult,
                        mybir.AluOpType.add,
                    )
                    if last:
                        eng = y_eng[y_cnt[0] % 3]
                        y_cnt[0] += 1
                        eng.dma_start(
                            y_d[:, mt, c0 - H:c0 - H + nb],
                            YS[:, mt, c0 - H:c0 - H + nb],
                        )

                def roll(groups, tag0, depth=2):
                    """Rolling depth-N software pipeline: at step s, group i
                    (for i in (s-depth, s]) runs its (s-i)-th segment of
                    KT/depth k-slots, oldest group first within each slot.
                    Hides the PE drain at accumulation-group boundaries."""
                    seg = KT // depth
                    n = len(groups)
                    gs = []
                    for s in range(n + depth - 1):
                        if s < n:
                            ps = gps_pool.tile([128, 512], F32, tag="gps",
                                               name=f"ps{tag0}_{s}")
                            gs.append((ps,) + tuple(groups[s]))
                        lo = max(0, s - depth + 1)
                        hi = min(s, n - 1)
                        for j in range(seg):
                            for i in range(lo, hi + 1):
                                g = gs[i]
                                mm_g(g[0], g[1], g[2], g[3], g[4],
                                     (s - i) * seg + j)
                        r = s - depth + 1
                        if 0 <= r < n:
                            epilogue_g(*gs[r])

                if not last:
                    row_blocks = ROW_BLOCKS_L[l]
                    for h in range(2):
                        # mtl-major so the first groups only gate on the
                        # first weight blocks while the rest stream in
                        groups = [(c0, nb, mtl, h)
                                  for mtl in range(4)
                                  for (c0, nb) in row_blocks]
                        roll(groups, f"{l}_{h}")

                    # refresh out-of-range halo rows with oob (data-driven;
                    # only the edge cores have nonzero masks)
                    for dt in range(DT):
                        nc.vector.copy_predicated(
                            S_out[:, dt, PAD:PAD + H],
                            mask_l[:, dt, :], fill_l[:, dt, :],
                        )
                        nc.vector.copy_predicated(
                            S_out[:, dt, PAD + B - H:PAD + B],
                            mask_r[:, dt, :], fill_r[:, dt, :],
                        )
                else:
                    # last layer: feature-tile-major so each mt's output
                    # chunks DMA out while later tiles compute
                    groups = [(c0, nb, mt % 4, mt // 4)
                              for mt in range(DT)
                              for (c0, nb) in ROW_BLOCKS_L[l]]
                    roll(groups, f"{l}", depth=3)

    nc.compile()
    return nc


def _get_nc():
    if "nc" not in _CACHE:
        _CACHE["nc"] = _build()
    return _CACHE["nc"]


def kernel(X, Ws, bs, oob):
    global LAST_RESULTS
    X = np.ascontiguousarray(np.asarray(X, np.float32))
    Ws = np.ascontiguousarray(np.asarray(Ws, np.float32))
    bs = np.ascontiguousarray(np.asarray(bs, np.float32))
    oob = np.ascontiguousarray(np.asarray(oob, np.float32))

    nc = _get_nc()

    BF = ml_dtypes.bfloat16
    # host-side input prep (layout rearrangement + bf16 rounding).
    # WT[l, h, mtl, p, j, c] = Ws[l, PERM[j]*128 + p, h*512 + mtl*128 + c]
    # — partition-major per output-column block (6KB contiguous DRAM
    # rows), k-axis in the kernel's PERM consumption order.
    WT = np.ascontiguousarray(
        Ws.astype(BF).reshape(NL, KT, 128, 2, 4, 128)
        .transpose(0, 3, 4, 2, 1, 5)[:, :, :, :, PERM, :]
    )
    BS = np.ascontiguousarray(
        (0.5 * bs).reshape(NL, DT, 128).transpose(2, 0, 1).reshape(128, NL * DT)
    )
    oobB = oob.astype(BF)
    oobT = np.ascontiguousarray(oobB.reshape(DT, 128).T)  # [128, DT]
    fill_edge = np.repeat(oobT[:, :, None], H, axis=2)    # [128, DT, H]
    ones = np.ones((128, DT, H), np.uint8)
    zeros_m = np.zeros((128, DT, H), np.uint8)
    zeros = np.zeros((128, DT, H), BF)

    in_maps = []
    for c in range(NCORES):
        lo, hi = c * M - H, c * M + M + H
        xs = np.empty((B, DIM), BF)
        slo, shi = max(lo, 0), min(hi, N)
        xs[slo - lo:shi - lo] = X[slo:shi].astype(BF)
        if lo < 0:
            xs[0:-lo] = oobB
        if hi > N:
            xs[B - (hi - N):] = oobB
        xst = np.ascontiguousarray(
            xs.reshape(B, DT, 128).transpose(2, 1, 0))
        left_edge = c == 0
        right_edge = c == NCORES - 1
        in_maps.append({
            "XST": xst,
            "WT": WT,
            "BS": BS,
            "ML": ones if left_edge else zeros_m,
            "FL": fill_edge if left_edge else zeros,
            "MR": ones if right_edge else zeros_m,
            "FR": fill_edge if right_edge else zeros,
        })

    res = run_bass_kernel_spmd(nc, in_maps, list(range(NCORES)))
    LAST_RESULTS = res
    out = np.concatenate(
        [res.results[c]["YT"].transpose(2, 1, 0).reshape(M, DIM)
         for c in range(NCORES)],
        axis=0,
    )
    return out[None, :, :].astype(np.float32)


# revision 19
# speedup vs baseline: 1.0022x; 1.0022x over previous
"""DilatedCNN forward on 8 TRN2 NeuronCores.

Strategy: data-parallel over the sequence dim N with halo. Each core owns
M=1024 rows plus an 8-row halo on each side (8 = sum of dilations
[1,2,4,1]); with the halo, all four layers are computed fully locally —
no collectives. The activation state lives in SBUF *transposed*
(feature-major: [128 partitions = feature chunk, rows in the free dim]) so
that
  * the concat [X, X_left, X_right] is just three column-shifted views of
    the same buffer (shifts along the free dim are free),
  * the 3072-feature contraction has features on partitions as the
    TensorEngine requires for both operands,
  * each layer's output is again feature-major — ready to be the next
    layer's input with no data movement,
  * the per-feature bias is a per-partition scalar for the activation op.

All matmul operands are bf16 (HW-measured: 147ns/instr at nb=346 vs
159ns for fp32r; LDWEIGHTS halves to 97ns so it stays hidden). The
state is *stored* bf16 and ping-pongs between two buffers per layer
(S0 -> S1 -> S0 -> S1), so no per-layer rounding copy is needed and
epilogue writes never hazard against the same layer's matmul reads.
End-to-end rel err vs the fp32 reference: ~3e-3 (gate 2e-2).

DMA is packet-rate-bound (~600ns minimum for any 128-row transfer), so
weights are laid out partition-major on the host ([l, h, 128, KT, 512])
and each half-layer loads in 2-4 large-row chunks instead of 24
per-tile transfers. Weight SBUF double-buffers by layer parity; each
layer's tiles stream in a whole layer ahead of use.

Accumulation groups are issued as a rolling 2-deep software pipeline
(group i's k12..k23 interleave with group i+1's k0..k11), hiding the
one-slot PE drain at every accumulation-group boundary. The final layer
runs feature-tile-major and writes fp32 results to a staging buffer
whose 512-row chunks DMA out as each epilogue lands.

Out-of-range rows are refreshed with the `oob` vector between layers via
copy_predicated driven by per-core mask/fill inputs, so all 8 cores run
one identical program.
"""

import numpy as np
import ml_dtypes

import concourse.bacc as bacc
import concourse.mybir as mybir
import concourse.tile as tile
from concourse.bass_utils import run_bass_kernel_spmd

N, DIM, NL = 8192, 1024, 4
NCORES = 8
M = N // NCORES           # rows per core
H = 8                     # halo rows each side (sum of dilations)
PAD = 4                   # zero cols so shifted reads stay in-bounds
B = M + 2 * H             # 1040 buffer rows
FB = PAD + B + PAD        # 1048 free-dim cols of the state buffer
DIL = [1, 2, 4, 1]
KT = 3 * DIM // 128       # 24 contraction tiles
DT = DIM // 128           # 8 feature tiles
# Per-layer compute windows (rows [start, start+size) of the B-row buffer),
# shrinking by the dilation each layer.
ROW_BLOCKS_L = [
    [(1, 346), (347, 346), (693, 346)],   # layer 1: rows [1, 1039)
    [(3, 346), (349, 344), (693, 344)],   # layer 2: rows [3, 1037)
    [(7, 342), (349, 342), (691, 342)],   # layer 3: rows [7, 1033)
    [(8, 512), (520, 512)],               # layer 4: rows [8, 1032)
]
F32 = mybir.dt.float32
BF16 = mybir.dt.bfloat16

# Contraction order, dt-grouped: slot j contracts k-tile PERM[j], so the
# k-loop touches input slab dt only every third slot — X slab dt_i is
# first needed at slot 3i, tripling the head's supply slack. PSUM
# accumulation is order-agnostic. The host lays the weight k-axis out in
# this order, so slot j reads weight row j and chunked weight DMAs
# arrive exactly in consumption order.
PERM = [(j % 3) * DT + j // 3 for j in range(KT)]

_CACHE = {}
LAST_RESULTS = None  # test harness reads exec_time_ns from here


def _build():
    nc = bacc.Bacc("TRN2", target_bir_lowering=False, debug=False)

    xs_d = nc.dram_tensor("XST", [128, DT, B], BF16, kind="ExternalInput")
    w_d = nc.dram_tensor("WT", [NL, 2, 4, 128, KT, 128], BF16,
                         kind="ExternalInput")
    b_d = nc.dram_tensor("BS", [128, NL * DT], F32, kind="ExternalInput")
    ml_d = nc.dram_tensor("ML", [128, DT, H], mybir.dt.uint8, kind="ExternalInput")
    fl_d = nc.dram_tensor("FL", [128, DT, H], BF16, kind="ExternalInput")
    mr_d = nc.dram_tensor("MR", [128, DT, H], mybir.dt.uint8, kind="ExternalInput")
    fr_d = nc.dram_tensor("FR", [128, DT, H], BF16, kind="ExternalInput")
    y_d = nc.dram_tensor("YT", [128, DT, M], F32, kind="ExternalOutput")

    with tile.TileContext(nc) as tc:
        with (
            tc.tile_pool(name="state", bufs=1) as state_pool,
            tc.tile_pool(name="wpool", bufs=1) as w_pool,
            tc.tile_pool(name="const", bufs=1) as const_pool,
            tc.tile_pool(name="tmp", bufs=4) as tmp_pool,
            tc.tile_pool(name="gps", bufs=6, space="PSUM") as gps_pool,
        ):
            S0 = state_pool.tile([128, DT, FB], BF16)  # even layers' input
            S1 = state_pool.tile([128, DT, FB], BF16)  # odd layers' input
            YS = state_pool.tile([128, DT, M], F32)    # final fp32 output

            # zero the PAD columns once; epilogues never touch them
            for Sb in (S0, S1):
                nc.gpsimd.memset(Sb[:, :, 0:PAD], 0.0)
                nc.gpsimd.memset(Sb[:, :, PAD + B:FB], 0.0)

            # ---- input prologue. All DMA queues share one AXI port
            # (~310 GB/s aggregate), and the sync/scalar queues are
            # hardware-DGE with ~one transfer in flight (each DMA trickles
            # out 2-4us apart), while gpsimd's software-DGE queue pipelines
            # at full port speed. So: everything head-critical rides gpsimd
            # in exact first-need order (first weight block's chunks
            # interleaved with the X slabs); sync/scalar only carry one
            # early X slab each plus the tiny consts.
            w_tiles = {}
            for l in range(NL):
                for h in range(2):
                    for mtl in range(4):
                        w_tiles[(l, h, mtl)] = w_pool.tile(
                            [128, KT, 128], BF16,
                            tag=f"w{l % 2}_{h}_{mtl}",
                            name=f"w{l}_{h}_{mtl}")

            nc.sync.dma_start(S0[:, 0, PAD:PAD + B], xs_d[:, 0, :])
            nc.scalar.dma_start(S0[:, 1, PAD:PAD + B], xs_d[:, 1, :])

            w000 = w_tiles[(0, 0, 0)]
            gp_seq = [
                lambda: nc.gpsimd.dma_start(w000[:, 0:8, :],
                                            w_d[0, 0, 0, :, 0:8, :]),
                lambda: nc.gpsimd.dma_start(S0[:, 2, PAD:PAD + B],
                                            xs_d[:, 2, :]),
                lambda: nc.gpsimd.dma_start(S0[:, 3, PAD:PAD + B],
                                            xs_d[:, 3, :]),
                lambda: nc.gpsimd.dma_start(w000[:, 8:16, :],
                                            w_d[0, 0, 0, :, 8:16, :]),
                lambda: nc.gpsimd.dma_start(S0[:, 4, PAD:PAD + B],
                                            xs_d[:, 4, :]),
                lambda: nc.gpsimd.dma_start(S0[:, 5, PAD:PAD + B],
                                            xs_d[:, 5, :]),
                lambda: nc.gpsimd.dma_start(w000[:, 16:KT, :],
                                            w_d[0, 0, 0, :, 16:KT, :]),
                lambda: nc.gpsimd.dma_start(S0[:, 6, PAD:PAD + B],
                                            xs_d[:, 6, :]),
                lambda: nc.gpsimd.dma_start(S0[:, 7, PAD:PAD + B],
                                            xs_d[:, 7, :]),
            ]
            for f in gp_seq:
                f()

            bs_t = const_pool.tile([128, NL * DT], F32)
            mask_l = const_pool.tile([128, DT, H], mybir.dt.uint8)
            fill_l = const_pool.tile([128, DT, H], BF16)
            mask_r = const_pool.tile([128, DT, H], mybir.dt.uint8)
            fill_r = const_pool.tile([128, DT, H], BF16)
            nc.scalar.dma_start(bs_t[:], b_d[:])
            nc.scalar.dma_start(mask_l[:], ml_d[:])
            nc.scalar.dma_start(fill_l[:], fl_d[:])
            nc.scalar.dma_start(mask_r[:], mr_d[:])
            nc.scalar.dma_start(fill_r[:], fr_d[:])

            # Remaining weight blocks stream on gpsimd in need order.
            # Weights double-buffer by layer parity ([128, KT, 128] per
            # (l, h, mtl), 6KB DRAM rows): layer l+1's tiles reuse layer
            # l-1's tags, so each load only waits on matmuls a whole layer
            # back and streams in during layer l.
            for l in range(NL):
                for h in range(2):
                    for mtl in range(4):
                        if (l, h, mtl) == (0, 0, 0):
                            continue
                        nc.gpsimd.dma_start(w_tiles[(l, h, mtl)][:],
                                            w_d[l, h, mtl])

            # ---- layers ----
            y_eng = [nc.sync, nc.scalar, nc.gpsimd]
            y_cnt = [0]
            for l, d in enumerate(DIL):
                last = l == NL - 1
                S_in = S0 if l % 2 == 0 else S1
                S_out = S1 if l % 2 == 0 else S0

                def mm_g(ps, c0, nb, mtl, h, j):
                    kt = PERM[j]
                    dt = kt % DT
                    grp = kt // DT
                    sh = 0 if grp == 0 else (-d if grp == 1 else d)
                    nc.tensor.matmul(
                        ps[:, 0:nb],
                        w_tiles[(l, h, mtl)][:, j, :],
                        S_in[:, dt, PAD + c0 + sh:PAD + c0 + sh + nb],
                        start=(j == 0),
                        stop=(j == KT - 1),
                    )

                def epilogue_g(ps, c0, nb, mtl, h):
                    mt = h * 4 + mtl
                    tmp = tmp_pool.tile([128, 512], F32, tag="tmp",
                                        name=f"tmp{l}_{h}_{c0}_{mtl}")
                    nc.scalar.activation(
                        tmp[:, 0:nb],
                        ps[:, 0:nb],
                        mybir.ActivationFunctionType.Relu,
                        bias=bs_t[:, l * DT + mt:l * DT + mt + 1],
                        scale=0.5,
                    )
                    # out = 0.5*S_in + relu(0.5*cat@W + 0.5*b)
                    if last:
                        out_ap = YS[:, mt, c0 - H:c0 - H + nb]
                    else:
                        out_ap = S_out[:, mt, PAD + c0:PAD + c0 + nb]
                    nc.vector.scalar_tensor_tensor(
                        out_ap,
                        S_in[:, mt, PAD + c0:PAD + c0 + nb],
                        0.5,
                        tmp[:, 0:nb],
                        mybir.AluOpType.mult,
                        mybir.AluOpType.add,
                    )
                    if last:
                        eng = y_eng[y_cnt[0] % 3]
                        y_cnt[0] += 1
                        eng.dma_start(
                            y_d[:, mt, c0 - H:c0 - H + nb],
                            YS[:, mt, c0 - H:c0 - H + nb],
                        )

                def roll(groups, tag0, depth=2):
                    """Rolling depth-N software pipeline: at step s, group i
                    (for i in (s-depth, s]) runs its (s-i)-th segment of
                    KT/depth k-slots, oldest group first within each slot.
                    Hides the PE drain at accumulation-group boundaries."""
                    seg = KT // depth
                    n = len(groups)
                    gs = []
                    for s in range(n + depth - 1):
                        if s < n:
                            ps = gps_pool.tile([128, 512], F32, tag="gps",
                                               name=f"ps{tag0}_{s}")
                            gs.append((ps,) + tuple(groups[s]))
                        lo = max(0, s - depth + 1)
                        hi = min(s, n - 1)
                        for j in range(seg):
                            for i in range(lo, hi + 1):
                                g = gs[i]
                                mm_g(g[0], g[1], g[2], g[3], g[4],
                                     (s - i) * seg + j)
                        r = s - depth + 1
                        if 0 <= r < n:
                            epilogue_g(*gs[r])

                if not last:
                    row_blocks = ROW_BLOCKS_L[l]
                    for h in range(2):
                        # mtl-major so the first groups only gate on the
                        # first weight blocks while the rest stream in
                        groups = [(c0, nb, mtl, h)
                                  for mtl in range(4)
                                  for (c0, nb) in row_blocks]
                        roll(groups, f"{l}_{h}")

                    # refresh out-of-range halo rows with oob (data-driven;
                    # only the edge cores have nonzero masks)
                    for dt in range(DT):
                        nc.vector.copy_predicated(
                            S_out[:, dt, PAD:PAD + H],
                            mask_l[:, dt, :], fill_l[:, dt, :],
                        )
                        nc.vector.copy_predicated(
                            S_out[:, dt, PAD + B - H:PAD + B],
                            mask_r[:, dt, :], fill_r[:, dt, :],
                        )
                else:
                    # last layer: feature-tile-major so each mt's output
                    # chunks DMA out while later tiles compute
                    groups = [(c0, nb, mt % 4, mt // 4)
                              for mt in range(DT)
                              for (c0, nb) in ROW_BLOCKS_L[l]]
                    roll(groups, f"{l}")

    nc.compile()
    return nc


def _get_nc():
    if "nc" not in _CACHE:
        _CACHE["nc"] = _build()
    return _CACHE["nc"]


def kernel(X, Ws, bs, oob):
    global LAST_RESULTS
    X = np.ascontiguousarray(np.asarray(X, np.float32))
    Ws = np.ascontiguousarray(np.asarray(Ws, np.float32))
    bs = np.ascontiguousarray(np.asarray(bs, np.float32))
    oob = np.ascontiguousarray(np.asarray(oob, np.float32))

    nc = _get_nc()

    BF = ml_dtypes.bfloat16
    # host-side input prep (layout rearrangement + bf16 rounding).
    # WT[l, h, mtl, p, j, c] = Ws[l, PERM[j]*128 + p, h*512 + mtl*128 + c]
    # — partition-major per output-column block (6KB contiguous DRAM
    # rows), k-axis in the kernel's PERM consumption order.
    WT = np.ascontiguousarray(
        Ws.astype(BF).reshape(NL, KT, 128, 2, 4, 128)
        .transpose(0, 3, 4, 2, 1, 5)[:, :, :, :, PERM, :]
    )
    BS = np.ascontiguousarray(
        (0.5 * bs).reshape(NL, DT, 128).transpose(2, 0, 1).reshape(128, NL * DT)
    )
    oobB = oob.astype(BF)
    oobT = np.ascontiguousarray(oobB.reshape(DT, 128).T)  # [128, DT]
    fill_edge = np.repeat(oobT[:, :, None], H, axis=2)    # [128, DT, H]
    ones = np.ones((128, DT, H), np.uint8)
    zeros_m = np.zeros((128, DT, H), np.uint8)
    zeros = np.zeros((128, DT, H), BF)

    in_maps = []
    for c in range(NCORES):
        lo, hi = c * M - H, c * M + M + H
        xs = np.empty((B, DIM), BF)
        slo, shi = max(lo, 0), min(hi, N)
        xs[slo - lo:shi - lo] = X[slo:shi].astype(BF)
        if lo < 0:
            xs[0:-lo] = oobB
        if hi > N:
            xs[B - (hi - N):] = oobB
        xst = np.ascontiguousarray(
            xs.reshape(B, DT, 128).transpose(2, 1, 0))
        left_edge = c == 0
        right_edge = c == NCORES - 1
        in_maps.append({
            "XST": xst,
            "WT": WT,
            "BS": BS,
            "ML": ones if left_edge else zeros_m,
            "FL": fill_edge if left_edge else zeros,
            "MR": ones if right_edge else zeros_m,
            "FR": fill_edge if right_edge else zeros,
        })

    res = run_bass_kernel_spmd(nc, in_maps, list(range(NCORES)))
    LAST_RESULTS = res
    out = np.concatenate(
        [res.results[c]["YT"].transpose(2, 1, 0).reshape(M, DIM)
         for c in range(NCORES)],
        axis=0,
    )
    return out[None, :, :].astype(np.float32)
